# revision 3
# baseline (speedup 1.0000x reference)
"""FCOS post-processing (class-spec NMS keep mask + location targets) on
8 Trainium2 NeuronCores.

Strategy
--------
The class-offset trick in the reference makes cross-class IoU exactly 0
(boxes of different classes live in disjoint coordinate intervals), so
greedy NMS decomposes into ~80 independent per-class problems. The host
groups eligible boxes by class in greedy order (score desc, index asc)
and packs per-class pairwise operand blocks into an SPMD-uniform slot
template (8 cores x NCLS slots; slots paired onto partition offsets
{0,64} when both fit in 64 partitions). The device computes, per pair,
the fp32 suppression predicate

    S[a,b] = (a precedes b) & (inter > 0.5*union)

with full-width DVE ops (bit-identical arithmetic to the reference's
IoU, validated: the multiply-compare equals the divide-compare for any
realizable margin), then resolves greedy NMS with a fixpoint
    keep <- (S^T @ keep == 0)
iterated T times via tiny per-slot PE matmuls (greedy == fixpoint limit;
depth is the longest suppression chain, ~1-2 for real data). The device
also emits S itself and the T-1 iterate so the host can verify
convergence and, in the (never-observed) non-converged case, finish the
scan exactly on the host from the device-computed S. FCOS location
targets are elementwise and computed alongside on the same cores.

kernel() accepts FULL inputs and returns the FULL (keep_mask, pred_out)
tuple, matching the reference bit-for-bit on the keep mask.
"""

from contextlib import ExitStack

import numpy as np

F32 = np.float32
P = 128
NCORES = 8
T_ITERS = 5

# ----------------------------------------------------------------------
# host-side planning / packing
# ----------------------------------------------------------------------


def _plan_nms(scores, class_ids):
    """Slot template + member assignment. None if a class exceeds 128."""
    N = scores.shape[0]
    first_max = scores.max()
    amax = int(scores.argmax())
    elig = scores < first_max
    elig[amax] = True

    order = np.lexsort((np.arange(N), -scores))  # score desc, idx asc
    per_class = {}
    for idx in order:
        if not elig[idx]:
            continue
        c = int(class_ids[idx])
        per_class.setdefault(c, []).append(int(idx))

    members = sorted(per_class.values(), key=len, reverse=True)
    if not members:
        members = [[]]
    if len(members[0]) > P:
        return None, elig

    NCLS = -(-len(members) // NCORES)
    ssize = []
    for t in range(NCLS):
        band = members[t * NCORES:(t + 1) * NCORES]
        mx = max(len(m) for m in band)
        ssize.append(max(4, -(-mx // 4) * 4))
    # pair slots into partition groups; PE base partition must be 0/32/64
    po = [0] * NCLS
    og = [0] * NCLS
    groups = []
    i, j = 0, NCLS - 1
    while i <= j:
        if i != j and ssize[i] <= 64 and ssize[j] <= 64:
            groups.append([i, j])
            i += 1
            j -= 1
        else:
            groups.append([i])
            i += 1
    col = 0
    for g in groups:
        po[g[0]] = 0
        if len(g) == 2:
            po[g[1]] = 64
        for t in g:
            og[t] = col
        col += max(ssize[t] for t in g)
    F = max(4, -(-col // 4) * 4)

    assign = [[None] * NCLS for _ in range(NCORES)]
    for r, m in enumerate(members):
        assign[r % NCORES][r // NCORES] = np.asarray(m, dtype=np.int64)

    return {"assign": assign, "ssize": ssize, "po": po, "og": og,
            "F": F, "NCLS": NCLS}, elig


def _pack_nms_inputs(plan, bo, areas):
    F = plan["F"]
    po, og = plan["po"], plan["og"]
    outs = []
    for k in range(NCORES):
        t = np.zeros((P, 10 * F), F32)
        v = [t[:, c * F:(c + 1) * F] for c in range(10)]
        X1I, Y1I, X2I, Y2I, X1J, Y1J, X2J, Y2J, SUMA, MASK = v
        for slot in range(plan["NCLS"]):
            idxs = plan["assign"][k][slot]
            if idxs is None or len(idxs) == 0:
                continue
            n = len(idxs)
            sp = slice(po[slot], po[slot] + n)
            sc = slice(og[slot], og[slot] + n)
            bx = bo[idxs]
            ar = areas[idxs]
            X1I[sp, sc] = bx[None, :, 0]
            Y1I[sp, sc] = bx[None, :, 1]
            X2I[sp, sc] = bx[None, :, 2]
            Y2I[sp, sc] = bx[None, :, 3]
            X1J[sp, sc] = bx[:, None, 0]
            Y1J[sp, sc] = bx[:, None, 1]
            X2J[sp, sc] = bx[:, None, 2]
            Y2J[sp, sc] = bx[:, None, 3]
            SUMA[sp, sc] = ar[:, None] + ar[None, :]
            jl = np.arange(n)
            MASK[sp, sc] = (jl[:, None] < jl[None, :]).astype(F32)
        outs.append(t)
    return outs


def _pack_fcos_inputs(locations, deltas):
    L = locations.shape[0]
    per = -(-L // NCORES)
    per = max(P, -(-per // P) * P)
    TW = per // P
    outs = []
    for k in range(NCORES):
        t = np.zeros((P, 6 * TW), F32)
        lo = k * per
        hi = min(L, lo + per)
        n = max(0, hi - lo)
        buf = np.ones((per, 6), F32)   # pad=1 keeps centerness finite
        buf[:, 0:2] = 0.0
        if n > 0:
            buf[:n, 0:2] = locations[lo:hi]
            buf[:n, 2:6] = deltas[lo:hi]
        for c in range(6):
            t[:, c * TW:(c + 1) * TW] = buf[:, c].reshape(P, TW)
        outs.append(t)
    return outs, TW, per


def _unpack_keep(plan, keep_outs, N):
    keep = np.zeros(N, bool)
    po = plan["po"]
    for k in range(NCORES):
        for slot in range(plan["NCLS"]):
            idxs = plan["assign"][k][slot]
            if idxs is None or len(idxs) == 0:
                continue
            p0 = po[slot]
            keep[idxs] = keep_outs[k][p0:p0 + len(idxs), slot] > 0.5
    return keep


def _unpack_fcos(fcos_outs, TW, L):
    rows = []
    for k in range(NCORES):
        r = np.empty((P * TW, 5), F32)
        for c in range(5):
            r[:, c] = fcos_outs[k][:, c * TW:(c + 1) * TW].reshape(-1)
        rows.append(r)
    return np.concatenate(rows, axis=0)[:L]


def _host_greedy_from_s(plan, s_outs, N):
    """Exact greedy scan from the device-computed S blocks (safety net)."""
    keep = np.zeros(N, bool)
    po, og = plan["po"], plan["og"]
    for k in range(NCORES):
        for slot in range(plan["NCLS"]):
            idxs = plan["assign"][k][slot]
            if idxs is None or len(idxs) == 0:
                continue
            n = len(idxs)
            Sb = s_outs[k][po[slot]:po[slot] + n,
                           og[slot]:og[slot] + n] > 0.5
            kp = np.ones(n, bool)
            for t in range(n):
                if kp[t]:
                    kp &= ~Sb[t]
                    kp[t] = True
            keep[idxs] = kp
    return keep


# ----------------------------------------------------------------------
# device kernel
# ----------------------------------------------------------------------

_CACHE = {}


def _build(F, NCLS, TW, ssize, po, og, T, stride_f):
    import concourse.bacc as bacc
    import concourse.tile as tile
    from concourse import mybir

    ALU = mybir.AluOpType
    DT = mybir.dt.float32

    nc = bacc.Bacc("TRN2", target_bir_lowering=False, debug=False,
                   num_devices=NCORES)
    nms_in = nc.dram_tensor("nms_in", [P, 10 * F], DT, kind="ExternalInput")
    fcos_in = nc.dram_tensor("fcos_in", [P, 6 * TW], DT,
                             kind="ExternalInput")
    keep_out = nc.dram_tensor("keep_out", [P, NCLS], DT,
                              kind="ExternalOutput")
    keep_prev = nc.dram_tensor("keep_prev", [P, NCLS], DT,
                               kind="ExternalOutput")
    s_out = nc.dram_tensor("s_out", [P, F], DT, kind="ExternalOutput")
    fcos_out = nc.dram_tensor("fcos_out", [P, 5 * TW], DT,
                              kind="ExternalOutput")
    s = float(stride_f)

    with tile.TileContext(nc) as tc:
        with ExitStack() as ctx:
            main = ctx.enter_context(tc.tile_pool(name="main", bufs=1))
            scr = ctx.enter_context(tc.tile_pool(name="scr", bufs=1))
            psum = ctx.enter_context(
                tc.tile_pool(name="psum", bufs=8, space="PSUM"))

            # ---- S matrix ----
            nt = main.tile([P, 10 * F], DT)
            nc.sync.dma_start(out=nt[:], in_=nms_in[:])
            X1I = nt[:, 0 * F:1 * F]
            Y1I = nt[:, 1 * F:2 * F]
            X2I = nt[:, 2 * F:3 * F]
            Y2I = nt[:, 3 * F:4 * F]
            X1J = nt[:, 4 * F:5 * F]
            Y1J = nt[:, 5 * F:6 * F]
            X2J = nt[:, 6 * F:7 * F]
            Y2J = nt[:, 7 * F:8 * F]
            SUMA = nt[:, 8 * F:9 * F]
            MASK = nt[:, 9 * F:10 * F]

            t0 = scr.tile([P, F], DT, tag="t0")
            t1 = scr.tile([P, F], DT, tag="t1")
            u0 = scr.tile([P, F], DT, tag="u0")
            u1 = scr.tile([P, F], DT, tag="u1")
            S = main.tile([P, F], DT, tag="S")

            nc.vector.tensor_tensor(t0[:], X2I, X2J, ALU.min)
            nc.vector.tensor_tensor(t1[:], X1I, X1J, ALU.max)
            nc.vector.tensor_tensor(u0[:], Y2I, Y2J, ALU.min)
            nc.vector.tensor_tensor(u1[:], Y1I, Y1J, ALU.max)
            nc.vector.tensor_sub(t0[:], t0[:], t1[:])        # xi
            nc.vector.tensor_sub(u0[:], u0[:], u1[:])        # yi
            nc.vector.tensor_scalar(u1[:], u0[:], 0.0, None, ALU.max)
            # inter = relu(xi) * relu(yi)
            nc.vector.scalar_tensor_tensor(t1[:], t0[:], 0.0, u1[:],
                                           ALU.max, ALU.mult)
            nc.vector.tensor_sub(u0[:], SUMA, t1[:])         # union
            # pred = (union * 0.5) < inter
            nc.vector.scalar_tensor_tensor(t0[:], u0[:], 0.5, t1[:],
                                           ALU.mult, ALU.is_lt)
            nc.vector.tensor_mul(S[:], t0[:], MASK)
            nc.sync.dma_start(out=s_out[:], in_=S[:])

            # ---- fixpoint ----
            keep = main.tile([P, NCLS], DT, tag="keep")
            prev = main.tile([P, NCLS], DT, tag="prev")
            nc.vector.memset(keep[:], 1.0)
            for it in range(T):
                if it == T - 1:
                    nc.vector.tensor_copy(out=prev[:], in_=keep[:])
                for slot in range(NCLS):
                    ss = ssize[slot]
                    p0 = po[slot]
                    c0 = og[slot]
                    pt = psum.tile([P, 1], DT, tag="fp")
                    nc.tensor.matmul(
                        pt[p0:p0 + ss, :],
                        S[p0:p0 + ss, c0:c0 + ss],
                        keep[p0:p0 + ss, slot:slot + 1],
                        start=True, stop=True,
                    )
                    nc.vector.tensor_scalar(
                        keep[p0:p0 + ss, slot:slot + 1],
                        pt[p0:p0 + ss, :], 0.0, None, ALU.is_le)
            nc.sync.dma_start(out=keep_prev[:], in_=prev[:])
            nc.sync.dma_start(out=keep_out[:], in_=keep[:])

            # ---- FCOS ----
            ft = main.tile([P, 6 * TW], DT, tag="ft")
            nc.sync.dma_start(out=ft[:], in_=fcos_in[:])
            px = ft[:, 0 * TW:1 * TW]
            py = ft[:, 1 * TW:2 * TW]
            d0 = ft[:, 2 * TW:3 * TW]
            d1 = ft[:, 3 * TW:4 * TW]
            d2 = ft[:, 4 * TW:5 * TW]
            d3 = ft[:, 5 * TW:6 * TW]
            fo = main.tile([P, 5 * TW], DT, tag="fo")

            w0 = scr.tile([P, TW], DT, tag="w0")
            w1 = scr.tile([P, TW], DT, tag="w1")
            w2 = scr.tile([P, TW], DT, tag="w2")

            for (dd, pp, sgn, col) in ((d0, px, -s, 0), (d1, py, -s, 1),
                                       (d2, px, s, 2), (d3, py, s, 3)):
                nc.vector.tensor_scalar(w0[:], dd, 0.0, sgn, ALU.max,
                                        ALU.mult)
                nc.vector.tensor_add(fo[:, col * TW:(col + 1) * TW],
                                     w0[:], pp)

            nc.vector.tensor_tensor(w0[:], d0, d2, ALU.min)
            nc.vector.tensor_tensor(w1[:], d1, d3, ALU.min)
            nc.vector.tensor_mul(w2[:], w0[:], w1[:])        # num
            nc.vector.tensor_tensor(w0[:], d0, d2, ALU.max)
            nc.vector.tensor_tensor(w1[:], d1, d3, ALU.max)
            nc.vector.tensor_mul(w0[:], w0[:], w1[:])        # den
            nc.vector.reciprocal(w1[:], w0[:])
            nc.vector.tensor_mul(w1[:], w2[:], w1[:])        # num/den
            nc.scalar.activation(w2[:], w1[:],
                                 mybir.ActivationFunctionType.Sqrt)
            nc.vector.tensor_scalar(w0[:], d0, -1.0, None, ALU.is_equal)
            nc.vector.scalar_tensor_tensor(w0[:], d1, -1.0, w0[:],
                                           ALU.is_equal, ALU.logical_and)
            nc.vector.scalar_tensor_tensor(w0[:], d2, -1.0, w0[:],
                                           ALU.is_equal, ALU.logical_and)
            nc.vector.scalar_tensor_tensor(w0[:], d3, -1.0, w0[:],
                                           ALU.is_equal, ALU.logical_and)
            nc.vector.memset(w1[:], -1.0)
            e8 = scr.tile([P, TW], mybir.dt.uint8, tag="e8")
            nc.vector.tensor_copy(out=e8[:], in_=w0[:])
            co = fo[:, 4 * TW:5 * TW]
            nc.vector.tensor_copy(out=co, in_=w2[:])
            nc.vector.copy_predicated(out=co, mask=e8[:], data=w1[:])
            nc.sync.dma_start(out=fcos_out[:], in_=fo[:])

    nc.compile()
    return nc


# ----------------------------------------------------------------------
# host fallback (pathological inputs only: class > 128 boxes, negative
# coordinates breaking the class-offset separation, or no trn devices)
# ----------------------------------------------------------------------


def _host_nms_reference(bo, scores):
    N = scores.shape[0]
    x1, y1, x2, y2 = bo[:, 0], bo[:, 1], bo[:, 2], bo[:, 3]
    areas = (x2 - x1) * (y2 - y1)
    first_max = scores.max()
    amax = int(scores.argmax())
    elig = scores < first_max
    elig[amax] = True
    order = np.lexsort((np.arange(N), -scores))
    keep = np.zeros(N, bool)
    sup = np.zeros(N, bool)
    for j in order:
        if not elig[j] or sup[j]:
            continue
        keep[j] = True
        xi = np.minimum(x2[j], x2) - np.maximum(x1[j], x1)
        yi = np.minimum(y2[j], y2) - np.maximum(y1[j], y1)
        inter = (np.maximum(xi, F32(0)) * np.maximum(yi, F32(0))).astype(F32)
        union = ((areas[j] + areas) - inter).astype(F32)
        iou = (inter / union).astype(F32)
        sup |= iou > F32(0.5)
    return keep


def _host_fcos(locations, deltas, s):
    px, py = locations[:, 0], locations[:, 1]
    cl = np.maximum(deltas, F32(0.0))
    pred = np.stack([
        (px - (cl[:, 0] * s).astype(F32)).astype(F32),
        (py - (cl[:, 1] * s).astype(F32)).astype(F32),
        (px + (cl[:, 2] * s).astype(F32)).astype(F32),
        (py + (cl[:, 3] * s).astype(F32)).astype(F32),
    ], axis=1)
    lrmin = np.minimum(deltas[:, 0], deltas[:, 2])
    tbmin = np.minimum(deltas[:, 1], deltas[:, 3])
    lrmax = np.maximum(deltas[:, 0], deltas[:, 2])
    tbmax = np.maximum(deltas[:, 1], deltas[:, 3])
    cent = np.sqrt(((lrmin * tbmin) / (lrmax * tbmax)).astype(F32))
    cent = np.where((deltas == -1.0).all(axis=1), F32(-1.0),
                    cent).astype(F32)
    return np.concatenate([pred, cent[:, None]], axis=1).astype(F32)


# ----------------------------------------------------------------------
# entry point
# ----------------------------------------------------------------------


def kernel(boxes, scores, class_ids, locations, deltas, stride):
    boxes = np.ascontiguousarray(np.asarray(boxes, F32))
    scores = np.ascontiguousarray(np.asarray(scores, F32))
    class_ids = np.asarray(class_ids)
    locations = np.ascontiguousarray(np.asarray(locations, F32))
    deltas = np.ascontiguousarray(np.asarray(deltas, F32))
    s = float(np.asarray(stride))

    N = boxes.shape[0]
    L = locations.shape[0]

    # exact same offset-box arithmetic as the reference (fp32)
    maxc = boxes.max()
    offs = (class_ids.astype(F32) * F32(maxc + F32(1.0))).astype(F32)
    bo = (boxes + offs[:, None]).astype(F32)
    areas = ((bo[:, 2] - bo[:, 0]) * (bo[:, 3] - bo[:, 1])).astype(F32)

    plan, elig = _plan_nms(scores, class_ids)
    separable = bool(boxes.min() >= 0) and np.isfinite(boxes).all()

    if plan is None or not separable:
        keep = _host_nms_reference(bo, scores)
        return keep, _host_fcos(locations, deltas, F32(s))

    nms_ins = _pack_nms_inputs(plan, bo, areas)
    fcos_ins, TW, _per = _pack_fcos_inputs(locations, deltas)

    key = (plan["F"], plan["NCLS"], TW, tuple(plan["ssize"]),
           tuple(plan["po"]), tuple(plan["og"]), T_ITERS, s)
    nc = _CACHE.get(key)
    if nc is None:
        nc = _build(plan["F"], plan["NCLS"], TW, plan["ssize"],
                    plan["po"], plan["og"], T_ITERS, s)
        _CACHE[key] = nc

    from concourse.bass_utils import run_bass_kernel_spmd
    in_maps = [{"nms_in": nms_ins[k], "fcos_in": fcos_ins[k]}
               for k in range(NCORES)]
    res = run_bass_kernel_spmd(nc, in_maps, core_ids=list(range(NCORES)))

    keep_outs = [res.results[k]["keep_out"] for k in range(NCORES)]
    prev_outs = [res.results[k]["keep_prev"] for k in range(NCORES)]
    conv = all(np.array_equal(a, b)
               for a, b in zip(keep_outs, prev_outs))
    if conv:
        keep = _unpack_keep(plan, keep_outs, N)
    else:
        s_outs = [res.results[k]["s_out"] for k in range(NCORES)]
        keep = _host_greedy_from_s(plan, s_outs, N)

    fcos_outs = [res.results[k]["fcos_out"] for k in range(NCORES)]
    pred_out = _unpack_fcos(fcos_outs, TW, L)
    return keep, pred_out
